# revision 8
# baseline (speedup 1.0000x reference)
"""Dynamic-conv (CondConv-style) kernel for Trainium2, 8 NeuronCores.

Problem: for each sample b:
    se     = global-avg-pool(x[b])                     (256,)
    gates  = sigmoid(se @ route_w.T + route_b)         (8,)
    w_dyn  = (gates @ weight.T).reshape(256,256,3,3)   per-sample 3x3 conv kernel
    out[b] = conv2d(x[b], w_dyn, pad=1) + bias         (256,28,28)

Sharding: data-parallel over batch, 4 samples per core; the expert weight
bank and routing weights are replicated.

Per-core plan (v4 — ring discipline + w-granule pipeline):
  - sync HWDGE ring carries, strictly in FIFO order: one merged constant
    blob, x (2 DMAs, partition-major [128, s, t, 28, 28] bf16), then the 6
    bank DMAs ([128,16,384], 12.3KB/partition contiguous) ordered
    w1,w0,w2,w4,w3,w5 to match consumption. SDMA engines round-robin
    descriptors with no priority, so x must be fully enqueued before bank.
  - scalar HWDGE ring carries only the stage->wd re-gathers and the output
    stores, so re-gathers never queue behind bank traffic (v2's bug).
  - pool: fused copy+accumulate, samples 0,1 on DVE and 2,3 on ACT
    (separate scratch each), chasing the two x DMAs.
  - gates: tiny fp32 matmuls vs a 16x-replicated routing matrix ->
    128x64 logits; sigmoid (ACT) + block-diag mask -> stationary
    G[(q,e),(s,q)] = gate[s,e] * (q==q').
  - synthesis per khkw-triple granule w: rhs partition p=(q,e) carries 16
    ci-chunks of all 8 experts; one matmul vs G computes sum_e g*W_e for
    16 chunks x 4 samples; PSUM drains (ACT/DVE alternating, cast bf16)
    into a (128,8,384) stage; flat re-gathers put ci on partitions.
  - conv granule (sample-pair, w): 24 edge-clipped shifted matmuls
    accumulate (ci half, 3 khkw) into open PSUM tiles; granule w1 runs
    first so its full-coverage k=4 matmul initializes PSUM (start=True).
    Samples {2,3} replay the granules after {0,1} drain (PSUM capacity);
    synthesis of the next half is issued in between to keep PE fed.
  - drain adds bias (ACT) writing bf16; host casts to fp32.
"""

import os
from contextlib import ExitStack

import ml_dtypes
import numpy as np

import concourse.bacc as bacc
import concourse.bass as bass
import concourse.mybir as mybir
import concourse.tile as tile
from concourse.bass_utils import run_bass_kernel_spmd

FP32 = mybir.dt.float32
BF16 = mybir.dt.bfloat16
BF16_NP = ml_dtypes.bfloat16

N_CORES = 8
B, C_IN, H, W = 32, 256, 28, 28
NUM, C_OUT, K = 8, 256, 3
BS = B // N_CORES          # samples per core = 4
NQ = 16                    # ci chunks in the synthesis contraction
F = 2304                   # f = co_t*1152 + khkw*128 + co_lo
NWIN = 384                 # synthesis matmul free size
NCB = 259 + 64             # const blob cols: rwx 256 | rbx 1 | bias 2 | mask 64


def build_nc() -> bacc.Bacc:
    nc = bacc.Bacc("TRN2", target_bir_lowering=False, debug=False,
                   num_devices=N_CORES)

    # x host-packed partition-major: [p, s, t, h, w], channel = t*128+perm(p)
    x_d = nc.dram_tensor("x", [128, BS, 2, H, W], BF16, kind="ExternalInput")
    # bank[p=(q,e), w, cl, n] = W[e, ci=q*16+cl, f=w*384+n]; per-partition
    # contiguous so one DMA per w moves 12.3KB/partition runs.
    bank_d = nc.dram_tensor("bank", [128, 6, NQ, NWIN], BF16, kind="ExternalInput")
    cst_d = nc.dram_tensor("cst", [128, NCB], FP32, kind="ExternalInput")
    out_d = nc.dram_tensor("out", [BS, C_OUT, H, W], BF16, kind="ExternalOutput")

    WSEQ = (1, 0, 2, 4, 3, 5)   # issue order: k=4-bearing granule first/half

    with tile.TileContext(nc) as tc, ExitStack() as ctx:
        singles = ctx.enter_context(tc.tile_pool(name="singles", bufs=1))
        bankp = ctx.enter_context(tc.tile_pool(name="bankp", bufs=6))
        stagep = ctx.enter_context(tc.tile_pool(name="stagep", bufs=3))
        wdynp = ctx.enter_context(tc.tile_pool(name="wdynp", bufs=1))
        outp = ctx.enter_context(tc.tile_pool(name="outp", bufs=2))
        psS = ctx.enter_context(tc.tile_pool(name="psS", bufs=4, space="PSUM"))
        psC = ctx.enter_context(tc.tile_pool(name="psC", bufs=4, space="PSUM"))

        # ---- sync ring, strict FIFO: const blob, x (2 DMAs), bank (6 DMAs)
        cst = singles.tile([128, NCB], FP32)
        nc.sync.dma_start(out=cst, in_=cst_d[:])
        xall = singles.tile([128, BS, 2, H, W], BF16)
        for s in range(BS):
            nc.sync.dma_start(out=xall[:, s], in_=x_d[:, s])
        bkt = {}
        for w in WSEQ:
            bk = bankp.tile([128, NQ, NWIN], BF16, tag="bk", name=f"bk{w}")
            nc.sync.dma_start(out=bk, in_=bank_d[:, w])
            bkt[w] = bk

        rwx = cst[:, 0:256]
        rbx = cst[:, 256:257]
        biasT = cst[:, 257:259]
        mask = cst[:, 259:NCB]

        ones16 = singles.tile([128, NQ], FP32)
        nc.vector.memset(ones16, 1.0)
        onesb = singles.tile([128, NQ], BF16)
        nc.vector.memset(onesb, 1.0)
        junk = singles.tile([128, 18, W], BF16)
        nc.gpsimd.memset(junk, 0.5)
        psW = psS.tile([128, 504], FP32, tag="ps", name="warmps")
        for i in range(44):
            nc.tensor.matmul(psW[0:NQ, :], lhsT=onesb, rhs=junk,
                             start=True, stop=True)
        warm = singles.tile([128, 1], FP32)
        nc.scalar.activation(out=warm, in_=ones16[:, 0:1],
                             func=mybir.ActivationFunctionType.Sigmoid)
        nc.scalar.activation(out=warm, in_=warm,
                             func=mybir.ActivationFunctionType.Identity,
                             bias=warm, scale=1.0)
        nc.scalar.activation(out=warm, in_=warm,
                             func=mybir.ActivationFunctionType.Copy)

        # ---- pooled se: fused DVE copy+accumulate chasing the two x DMAs
        se = singles.tile([128, 2, BS], FP32)
        scrV = singles.tile([128, H, W], BF16)
        for s in range(BS):
            for t in range(2):
                nc.vector.tensor_scalar(
                    out=scrV, in0=xall[:, s, t],
                    scalar1=1.0, scalar2=None,
                    op0=mybir.AluOpType.mult,
                    op1=mybir.AluOpType.add,
                    accum_out=se[:, t, s:s + 1])

        # ---- gates -> block-diagonal stationary G (M is (s, q) sample-major)
        se_rep = singles.tile([128, 2, 64], FP32)
        for t in range(2):
            for s in range(BS):
                nc.vector.tensor_scalar(
                    out=se_rep[:, t, NQ * s:NQ * (s + 1)], in0=ones16,
                    scalar1=se[:, t, s:s + 1], scalar2=None,
                    op0=mybir.AluOpType.mult)
        L = psS.tile([128, 64], FP32, tag="ps", name="Lpsum")
        for t in range(2):
            nc.tensor.matmul(L, lhsT=rwx[:, 128 * t:128 * (t + 1)],
                             rhs=se_rep[:, t, :],
                             start=(t == 0), stop=(t == 1))
        g0 = singles.tile([128, 64], FP32)
        nc.scalar.activation(out=g0, in_=L,
                             func=mybir.ActivationFunctionType.Sigmoid,
                             bias=rbx, scale=1.0)
        G = singles.tile([128, 64], BF16)
        nc.vector.tensor_tensor(out=G, in0=g0, in1=mask, op=mybir.AluOpType.mult)

        # ---- w-granule pipeline: synth -> re-gather (scalar ring) -> conv
        wd2 = {(half, s): wdynp.tile([128, 2, 9 * 128], BF16,
                                     tag=f"wd{half}{s}", name=f"wd{half}{s}")
               for half in range(2) for s in range(BS)}
        pst = {}

        def synth(w):
            half, wloc = divmod(w, 3)
            stg = stagep.tile([128, 8, NWIN], BF16, tag="stg", name=f"stg{w}")
            for clp in range(8):
                # MMs for cl=clp (u=0) and cl=clp+8 (u=1) share one (128,384)
                # PSUM tile via partition halves; one drain covers both
                ps = psS.tile([128, NWIN], FP32, tag="ps", name=f"ps{w}_{clp}")
                nc.tensor.matmul(ps[0:64, :], lhsT=G, rhs=bkt[w][:, clp, :],
                                 start=True, stop=True)
                nc.tensor.matmul(ps[64:128, :], lhsT=G,
                                 rhs=bkt[w][:, 8 + clp, :],
                                 start=True, stop=True)
                dst = stg[:, clp, :]
                if clp % 2 == 0:
                    nc.scalar.activation(
                        out=dst, in_=ps,
                        func=mybir.ActivationFunctionType.Copy)
                else:
                    nc.vector.tensor_copy(out=dst, in_=ps)
            # stage partition 64u+16s+8t+q' holds ci=(8t+q')*16+clp+8u for clp
            # in the free dim; with the host-side ci permutation, partition
            # d=64u+8q'+clp of the conv stationary IS that channel. Flat
            # [64,384]<-[8,8*384] views keep each dst partition one contiguous
            # 768B run. Scalar ring: never queues behind x/bank.
            for s in range(BS):
                for t in range(2):
                    for u in range(2):
                        src = stg[64 * u + NQ * s + 8 * t:
                                  64 * u + NQ * s + 8 * t + 8]
                        nc.scalar.dma_start(
                            out=wd2[half, s][64 * u:64 * (u + 1), t,
                                             wloc * NWIN:(wloc + 1) * NWIN],
                            in_=src)

        def conv(w, sgroup):
            half, wloc = divmod(w, 3)
            first = wloc == 1            # k=4-bearing granule runs first
            last = wloc == 2
            for s in sgroup:
                for t in range(2):
                    ks = ((4, 3, 5) if first and t == 0
                          else range(3 * wloc, 3 * wloc + 3))
                    for k in ks:
                        kh, kw = divmod(k, 3)
                        lw = wd2[half, s][:, t, k * 128:(k + 1) * 128]
                        xlo, xhi = max(0, 1 - kw), min(W - 1, W - kw)
                        for c in range(2):
                            ylo = max(c * 14, 1 - kh)
                            yhi = min(c * 14 + 13, H - kh)
                            rhs = xall[:, s, t,
                                       ylo + kh - 1:yhi + kh,
                                       xlo + kw - 1:xhi + kw]
                            nc.tensor.matmul(
                                pst[half, s][c][:, ylo - c * 14:
                                                yhi + 1 - c * 14, xlo:xhi + 1],
                                lhsT=lw, rhs=rhs,
                                start=(first and t == 0 and k == 4),
                                stop=(last and t == 1 and k == 3 * wloc + 2),
                            )

        def alloc_pst(half, sgroup):
            for s in sgroup:
                pst[half, s] = [psC.tile([128, 14, W], FP32, tag="pc",
                                         name=f"pc{half}_{s}_{c}")
                                for c in range(2)]

        def drain(half, sgroup):
            for s in sgroup:
                ot = outp.tile([128, 2, 14, W], BF16, tag="ot",
                               name=f"ot{half}_{s}")
                for c in range(2):
                    nc.scalar.activation(
                        out=ot[:, c], in_=pst[half, s][c],
                        func=mybir.ActivationFunctionType.Identity,
                        bias=biasT[:, half:half + 1], scale=1.0)
                nc.sync.dma_start(
                    out=out_d[s, half * 128:(half + 1) * 128], in_=ot)

        S01, S23 = (0, 1), (2, 3)
        synth(1); synth(0)
        alloc_pst(0, S01); conv(1, S01); conv(0, S01)
        synth(2)
        conv(2, S01); drain(0, S01)
        synth(4)                        # next-half prefetch keeps PE fed
        alloc_pst(0, S23); conv(1, S23); conv(0, S23); conv(2, S23)
        drain(0, S23)
        synth(3)
        alloc_pst(1, S01); conv(4, S01); conv(3, S01)
        synth(5)
        conv(5, S01); drain(1, S01)
        alloc_pst(1, S23); conv(4, S23); conv(3, S23); conv(5, S23)
        drain(1, S23)
    nc.finalize()
    return nc


# partition d (within a 128-channel tile) holds channel perm[d]:
# d = 64u + 8q' + clp  <->  ci_lo = 16q' + 8u + clp
CI_PERM = np.array([(d % 64) // 8 * 16 + (d // 64) * 8 + d % 8
                    for d in range(128)])
CI_MAP = np.concatenate([CI_PERM, 128 + CI_PERM])


def _host_prep(route_w, route_b, weight, bias):
    """Host-side layout transforms (pure numpy, replicated to every core)."""
    We = np.ascontiguousarray(weight.T).reshape(NUM, C_OUT, C_IN, K, K)
    Wf = We.transpose(0, 2, 1, 3, 4)            # [e, ci, co, kh, kw]
    Wf = Wf.reshape(NUM, C_IN, 2, 128, 9)       # [e, ci, co_t, co_lo, khkw]
    Wf = Wf.transpose(0, 1, 2, 4, 3)            # [e, ci, co_t, khkw, co_lo]
    Wf = Wf.reshape(NUM, C_IN, F)               # f = co_t*1152 + khkw*128 + co_lo
    Bk = Wf.reshape(NUM, NQ, NQ, 6, NWIN)       # [e, q, cl, w, n]
    bank = np.ascontiguousarray(
        Bk.transpose(1, 0, 3, 2, 4).reshape(128, 6, NQ, NWIN)).astype(BF16_NP)

    cst = np.empty((128, NCB), np.float32)
    cst[:, 0:256] = np.tile((route_w / (H * W)).T, (1, NQ))[CI_MAP] \
        .reshape(2, 128, 128).transpose(1, 0, 2).reshape(128, 256)
    cst[:, 256] = np.tile(route_b, NQ)
    cst[:, 257:259] = bias.reshape(2, 128).T
    # G column m = (s, q): q(m) = m % 16
    cst[:, 259:NCB] = (np.arange(128)[:, None] // 8
                       == np.arange(64)[None, :] % NQ)
    return bank, cst


def _ensure_ntff_hook():
    """Provide antenv.axon_hooks (absent in this image) so trace=True works.

    The boot script ships a ctypes NTFF hook but can only register it through
    antenv.axon_hooks; shim that module and register the hook ourselves.
    """
    import sys
    import types
    try:
        from antenv.axon_hooks import get_axon_ntff_profile_hook  # noqa: F401
        return
    except ImportError:
        pass
    try:
        import antenv
        from trn_agent_boot.trn_boot import _ntff_profile_via_ctypes
    except ImportError:
        return
    mod = types.ModuleType("antenv.axon_hooks")
    holder = {"hook": None}
    mod.set_axon_ntff_profile_hook = lambda h: holder.__setitem__("hook", h)
    mod.get_axon_ntff_profile_hook = lambda: holder["hook"]
    sys.modules["antenv.axon_hooks"] = mod
    antenv.axon_hooks = mod
    mod.set_axon_ntff_profile_hook(
        _ntff_profile_via_ctypes("/opt/axon/libaxon_pjrt.so"))


_NC_CACHE = None


def kernel(inputs, route_w, route_b, weight, bias):
    global _NC_CACHE
    inputs = np.asarray(inputs, dtype=np.float32)
    route_w = np.asarray(route_w, dtype=np.float32)
    route_b = np.asarray(route_b, dtype=np.float32)
    weight = np.asarray(weight, dtype=np.float32)
    bias = np.asarray(bias, dtype=np.float32)

    bank, cst = _host_prep(route_w, route_b, weight, bias)

    if _NC_CACHE is None:
        _NC_CACHE = build_nc()
    nc = _NC_CACHE

    shared = {"bank": bank, "cst": cst}
    # [B, 256, H, W] -> per-core [128, BS, 2, H, W]; channel = t*128+perm(p)
    x16 = inputs[:, CI_MAP].astype(BF16_NP).reshape(B, 2, 128, H, W)
    in_maps = []
    for c in range(N_CORES):
        xc = x16[BS * c:BS * (c + 1)]            # [BS, 2, 128, H, W]
        xc = np.ascontiguousarray(xc.transpose(2, 0, 1, 3, 4))
        in_maps.append({"x": xc, **shared})
    trace = bool(int(os.environ.get("KERNEL_TRACE", "0")))
    if trace:
        _ensure_ntff_hook()
    res = run_bass_kernel_spmd(
        nc, in_maps, core_ids=list(range(N_CORES)), trace=trace,
        tmpdir=os.environ.get("KERNEL_TMPDIR"),
    )
    out = np.concatenate([res.results[c]["out"] for c in range(N_CORES)],
                         axis=0).astype(np.float32)
    kernel.last_results = res
    return out


kernel.last_results = None
